# revision 15
# baseline (speedup 1.0000x reference)
"""RGCN (3 RelGraphConv layers + mean-pool + MLP + softmax) on 8 Trainium2 cores.

Strategy (dst-node sharding, data-parallel message passing):
  - Core c owns destination nodes [c*6250, (c+1)*6250), processed in 25 tiles
    of 256 nodes.
  - Host groups each core's incoming edges by (dst-tile, relation) plus a
    self-loop pseudo-relation (r=16, W_loop), and splits every group by source
    half (src < 32768 vs src >= 17232) into 128-slot columns so dma_gather's
    int16 indices can address the h table via two overlapping slices.
    The column layout (max over cores) is identical across cores -> one SPMD
    program serves all 8.
  - h is stored in fp16 (halves gather bandwidth); all PSUM accumulation is
    fp32.
  - Per dst tile: two dma_gather calls (A-table / B-table) fetch h[src] rows
    for all the tile's columns; a one-hot matrix H (DVE iota/is_equal, fp16)
    scatter-reduces via matmul A^T = G^T @ H per group into PSUM; after a
    PSUM->SBUF (cast to fp16) copy, agg += A @ W_r accumulates in PSUM
    (bias via rank-1 matmul); ReLU emits the core's h_next rows (fp16).
  - AllGather replicates h_next across cores between conv layers (2 total).
  - Layer-3 output feeds a weighted one-hot pooling matmul (weights 1/cnt
    host-computed) accumulated over all node subtiles -> [feat, graphs]
    partials; one AllReduce combines cores; the MLP runs transposed
    (h^T stays [feat, graphs]) in fp32; PE transpose + free-dim softmax
    produce the [128, 8] output (identical on every core).
"""

import sys

sys.path.insert(0, "/opt/trn_rl_repo")

import math
import numpy as np

from concourse import bass, bacc, mybir, tile
from concourse import bass_utils

F32 = mybir.dt.float32
F16 = mybir.dt.float16
I16 = mybir.dt.int16

TS = 256  # dst nodes per tile


class Cfg:
    def __init__(self, N, R, G, NC, cores, conv, mlp, split_cap=32768):
        self.N = N
        self.R = R
        self.G = G
        self.NC = NC
        self.cores = cores
        self.conv = conv
        self.mlp = mlp
        self.NPC = N // cores
        self.T = math.ceil(self.NPC / TS)
        self.split_cap = min(split_cap, N)  # table A = [0, split_cap)
        self.baseB = max(0, N - self.split_cap)  # table B = [baseB, N)


FULL_CFG = Cfg(
    N=50000, R=16, G=128, NC=8, cores=8,
    conv=[(128, 128), (128, 256), (256, 128)],
    mlp=[(128, 128), (128, 256), (256, 128)],
)


def _preprocess_edges(src, dst, rel, cfg):
    """Column layout (shared) + per-core slot arrays.

    Per tile column order: [A-cols of rel 0..R][B-cols of rel 0..R].
    Returns layout plus per-core idx (int16 table-local, 16-wrapped per call)
    and dstl (fp16 local dst id in tile, -1 pad).
    """
    C, NPC, T, R = cfg.cores, cfg.NPC, cfg.T, cfg.R
    NG = R + 1
    rows_t = np.minimum(NPC - np.arange(T) * TS, TS).astype(np.int64)

    owner = dst // NPC
    tloc = (dst % NPC) // TS
    isB = (src >= cfg.split_cap).astype(np.int64)
    # counts per (core, tile, half, rel)
    key_full = ((owner * T + tloc) * 2 + isB) * NG + rel
    cnt = np.bincount(key_full, minlength=C * T * 2 * NG).reshape(C, T, 2, NG)
    # self edges (r = R): node ids per tile are contiguous -> count per half
    for t in range(T):
        lo = t * TS
        for c in range(C):
            vs = c * NPC + lo + np.arange(int(rows_t[t]))
            nb = int((vs >= cfg.split_cap).sum())
            cnt[c, t, 0, R] = len(vs) - nb
            cnt[c, t, 1, R] = nb

    ncols = -(-cnt.max(axis=0) // 128)  # [T, 2, NG]
    # per-tile columns: A block then B block
    nA_t = ncols[:, 0, :].sum(axis=1)
    nB_t = ncols[:, 1, :].sum(axis=1)
    Ct_t = nA_t + nB_t
    tile_base = np.zeros(T + 1, np.int64)
    tile_base[1:] = np.cumsum(Ct_t)
    NCOL = int(tile_base[-1])

    # column start of each (t, half, rel) group-part
    colstart = np.zeros((T, 2, NG), np.int64)
    for t in range(T):
        cur = tile_base[t]
        for h in range(2):
            for r in range(NG):
                colstart[t, h, r] = cur
                cur += ncols[t, h, r]

    idx_po = np.zeros((C, 128, NCOL), np.int16)
    dstl_po = np.full((C, 128, NCOL), -1.0, np.float16)
    for c in range(C):
        m = owner == c
        es, ed, er = src[m], dst[m], rel[m]
        eb = (es >= cfg.split_cap).astype(np.int64)
        tl = (ed - c * NPC) // TS
        dl = (ed - c * NPC) % TS
        k = (tl * 2 + eb) * NG + er
        order = np.argsort(k, kind="stable")
        es, dl, k, eb = es[order], dl[order], k[order], eb[order]
        grp_start = np.searchsorted(k, np.arange(T * 2 * NG))
        j = np.arange(len(k)) - grp_start[k]
        kt, krem = k // (2 * NG), k % (2 * NG)
        kh, kr = krem // NG, krem % NG
        col = colstart[kt, kh, kr] + j // 128
        p = j % 128
        val = np.where(eb == 1, es - cfg.baseB, es).astype(np.int16)
        idx_po[c, p, col] = val
        dstl_po[c, p, col] = dl.astype(np.float16)
        # self edges
        for t in range(T):
            vl = np.arange(int(rows_t[t]))
            vg = c * NPC + t * TS + vl
            for h in range(2):
                sel = (vg >= cfg.split_cap) == (h == 1)
                if not sel.any():
                    continue
                vv, ll = vg[sel], vl[sel]
                cs = colstart[t, h, R]
                jj = np.arange(len(vv))
                vval = vv - cfg.baseB if h == 1 else vv
                idx_po[c, jj % 128, cs + jj // 128] = vval.astype(np.int16)
                dstl_po[c, jj % 128, cs + jj // 128] = ll.astype(np.float16)

    # 16-wrapped per-call index array: call (t, half) covers its column block
    TOT = NCOL * 128
    idx_w = np.zeros((C, 128, TOT // 16), np.int16)
    for c in range(C):
        for t in range(T):
            for h in range(2):
                if h == 0:
                    c0, ncol = tile_base[t], nA_t[t]
                else:
                    c0, ncol = tile_base[t] + nA_t[t], nB_t[t]
                if ncol == 0:
                    continue
                flat = idx_po[c, :, c0:c0 + ncol].T.ravel()  # j = col*128 + p
                blk = flat.reshape(-1, 16).T  # [16, n/16]
                o0 = int(c0) * 8  # c0*128/16
                idx_w[c, :, o0:o0 + len(flat) // 16] = np.tile(blk, (8, 1))

    layout = {
        "NCOL": NCOL,
        "ncols": ncols,            # [T, 2, NG]
        "colstart": colstart,      # [T, 2, NG]
        "tile_base": tile_base,
        "nA_t": nA_t, "nB_t": nB_t,
        "rows_t": rows_t,
        "Cmax": int(Ct_t.max()),
    }
    return layout, idx_w, dstl_po


def _pack_weights(inputs, cfg):
    R = cfg.R
    packed = {}
    for l, (di, do) in enumerate(cfg.conv):
        ks = di // 128
        Wp = np.zeros((ks, 128, (R + 1) * do), np.float16)
        Wr = np.asarray(inputs[f"W_rel{l}"], np.float32)
        Wl = np.asarray(inputs[f"W_loop{l}"], np.float32)
        for k in range(ks):
            for r in range(R):
                Wp[k, :, r * do:(r + 1) * do] = Wr[r, k * 128:(k + 1) * 128, :]
            Wp[k, :, R * do:(R + 1) * do] = Wl[k * 128:(k + 1) * 128, :]
        packed[f"Wcv{l}"] = Wp
        packed[f"bcv{l}"] = np.asarray(inputs[f"b{l}"], np.float16).reshape(1, do)
    for l, (di, do) in enumerate(cfg.mlp):
        ks = di // 128
        W = np.asarray(inputs[f"Wh{l}"], np.float32)
        packed[f"Wm{l}"] = W.reshape(ks, 128, do)
        nmt = -(-do // 128)
        bp = np.zeros((nmt, 128, 1), np.float32)
        b = np.asarray(inputs[f"bh{l}"], np.float32)
        for mi in range(nmt):
            seg = b[mi * 128:(mi + 1) * 128]
            bp[mi, :len(seg), 0] = seg
        packed[f"bm{l}"] = bp
    packed["Wcls"] = np.asarray(inputs["Wc"], np.float32).reshape(1, 128, cfg.NC)
    packed["bcls"] = np.asarray(inputs["bc"], np.float32).reshape(cfg.NC, 1)
    return packed


def _pool_arrays(graph_ids, cfg):
    """Per 128-node subtile: graph id and 1/cnt weight per node (fp16)."""
    C, NPC = cfg.cores, cfg.NPC
    ST = math.ceil(NPC / 128)
    cnts = np.bincount(graph_ids, minlength=cfg.G).astype(np.float64)
    wg = (1.0 / np.maximum(cnts, 1.0)).astype(np.float32)
    gid_po = np.full((C, 128, ST), -1.0, np.float16)
    wnd_po = np.zeros((C, 128, ST), np.float32)
    for c in range(C):
        for st in range(ST):
            nt = min(128, NPC - st * 128)
            if nt <= 0:
                continue
            v = c * NPC + st * 128 + np.arange(nt)
            gid_po[c, :nt, st] = graph_ids[v].astype(np.float16)
            wnd_po[c, :nt, st] = wg[graph_ids[v]]
    return gid_po, wnd_po


def build_program(cfg, layout, debug=False, timing=False):
    nc = bacc.Bacc(
        "TRN2", target_bir_lowering=False, debug=False,
        enable_asserts=False, num_devices=cfg.cores,
    )
    R, T, G, NC = cfg.R, cfg.T, cfg.G, cfg.NC
    NG = R + 1
    NCOL, Cmax = layout["NCOL"], layout["Cmax"]
    ncols, colstart = layout["ncols"], layout["colstart"]
    tile_base, nA_t, nB_t = layout["tile_base"], layout["nA_t"], layout["nB_t"]
    rows_t = layout["rows_t"]
    ST = math.ceil(cfg.NPC / 128)
    D0 = cfg.conv[0][0]

    h0 = nc.dram_tensor("h0", [cfg.N, D0], F16, kind="ExternalInput")
    idxT = nc.dram_tensor("idxw", [128, NCOL * 8], I16, kind="ExternalInput")
    dstlT = nc.dram_tensor("dstl", [128, NCOL], F16, kind="ExternalInput")
    gidT = nc.dram_tensor("gid", [128, ST], F16, kind="ExternalInput")
    wndT = nc.dram_tensor("wnd", [128, ST], F32, kind="ExternalInput")
    iotaT = nc.dram_tensor("iota", [128, max(Cmax, 1) * TS], F16, kind="ExternalInput")
    onesT = nc.dram_tensor("ones1", [1, 128], F16, kind="ExternalInput")
    idcT = nc.dram_tensor("idc", [NC, NC], F32, kind="ExternalInput")
    WcvT, bcvT = [], []
    for l, (di, do) in enumerate(cfg.conv):
        WcvT.append(nc.dram_tensor(f"Wcv{l}", [di // 128, 128, NG * do], F16,
                                   kind="ExternalInput"))
        bcvT.append(nc.dram_tensor(f"bcv{l}", [1, do], F16, kind="ExternalInput"))
    WmT, bmT = [], []
    for l, (di, do) in enumerate(cfg.mlp):
        WmT.append(nc.dram_tensor(f"Wm{l}", [di // 128, 128, do], F32,
                                  kind="ExternalInput"))
        bmT.append(nc.dram_tensor(f"bm{l}", [-(-do // 128), 128, 1], F32,
                                  kind="ExternalInput"))
    WclsT = nc.dram_tensor("Wcls", [1, 128, NC], F32, kind="ExternalInput")
    bclsT = nc.dram_tensor("bcls", [NC, 1], F32, kind="ExternalInput")
    outT = nc.dram_tensor("out", [G, NC], F32, kind="ExternalOutput")

    h_full = [h0]
    ag_in = []
    for l in range(2):
        do = cfg.conv[l][1]
        ag_in.append(nc.dram_tensor(f"agin{l}", [cfg.NPC, do], F16))
        h_full.append(nc.dram_tensor(f"hfull{l + 1}", [cfg.N, do], F16))
    pool_in = nc.dram_tensor("plin", [128, G], F32)
    pool_out = nc.dram_tensor("plout", [128, G], F32)
    dbg = {}
    if debug:
        dbg["h1"] = nc.dram_tensor("dbg_h1", [cfg.N, cfg.conv[0][1]], F16,
                                   kind="ExternalOutput")
        dbg["h2"] = nc.dram_tensor("dbg_h2", [cfg.N, cfg.conv[1][1]], F16,
                                   kind="ExternalOutput")
        dbg["pool"] = nc.dram_tensor("dbg_pool", [128, G], F32,
                                     kind="ExternalOutput")
        dbg["lg"] = nc.dram_tensor("dbg_lg", [G, NC], F32, kind="ExternalOutput")

    rg = [list(range(cfg.cores))]

    with tile.TileContext(nc) as tc:
        with (
            tc.tile_pool(name="const", bufs=1) as cp,
            tc.tile_pool(name="wp", bufs=1) as wp,
            tc.tile_pool(name="gp", bufs=2) as gp,
            tc.tile_pool(name="hp", bufs=2) as hp,
            tc.tile_pool(name="atp", bufs=2) as atp,
            tc.tile_pool(name="hnp", bufs=3) as hnp,
            tc.tile_pool(name="mp", bufs=2) as mp,
            tc.tile_pool(name="psA", bufs=2, space="PSUM") as psA,
            tc.tile_pool(name="psG", bufs=2, space="PSUM") as psG,
            tc.tile_pool(name="psP", bufs=1, space="PSUM") as psP,
        ):
            idx_sb = cp.tile([128, NCOL * 8], I16)
            nc.sync.dma_start(out=idx_sb[:], in_=idxT[:, :])
            dstl_sb = cp.tile([128, NCOL], F16)
            nc.sync.dma_start(out=dstl_sb[:], in_=dstlT[:, :])
            iota_sb = cp.tile([128, max(Cmax, 1) * TS], F16)
            nc.sync.dma_start(out=iota_sb[:], in_=iotaT[:, :])
            gid_sb = cp.tile([128, ST], F16)
            nc.sync.dma_start(out=gid_sb[:], in_=gidT[:, :])
            wnd_sb = cp.tile([128, ST], F32)
            nc.sync.dma_start(out=wnd_sb[:], in_=wndT[:, :])
            ones_sb = cp.tile([1, 128], F16)
            nc.sync.dma_start(out=ones_sb[:], in_=onesT[:, :])
            idc_sb = cp.tile([NC, NC], F32)
            nc.sync.dma_start(out=idc_sb[:], in_=idcT[:, :])

            Wsb, bsb = [], []
            for l, (di, do) in enumerate(cfg.conv):
                ks = di // 128
                Wk = []
                for k in range(ks):
                    w = wp.tile([128, NG * do], F16, tag=f"wcv{l}_{k}")
                    nc.sync.dma_start(out=w[:], in_=WcvT[l][k, :, :])
                    Wk.append(w)
                Wsb.append(Wk)
                b = wp.tile([1, do], F16, tag=f"bcv{l}")
                nc.sync.dma_start(out=b[:], in_=bcvT[l][:, :])
                bsb.append(b)
            Wm_sb, bm_sb = [], []
            for l, (di, do) in enumerate(cfg.mlp):
                ks = di // 128
                Wk = []
                for k in range(ks):
                    w = wp.tile([128, do], F32, tag=f"wm{l}_{k}")
                    nc.sync.dma_start(out=w[:], in_=WmT[l][k, :, :])
                    Wk.append(w)
                Wm_sb.append(Wk)
                nmt = -(-do // 128)
                bk = []
                for mi in range(nmt):
                    b = wp.tile([128, 1], F32, tag=f"bm{l}_{mi}")
                    nc.sync.dma_start(out=b[:], in_=bmT[l][mi, :, :])
                    bk.append(b)
                bm_sb.append(bk)
            Wcls_sb = wp.tile([128, NC], F32, tag="wcls")
            nc.sync.dma_start(out=Wcls_sb[:], in_=WclsT[0, :, :])
            bcls_sb = wp.tile([NC, 1], F32, tag="bcls")
            nc.sync.dma_start(out=bcls_sb[:], in_=bclsT[:, :])

            pool_ps = None

            for l, (di, do) in enumerate(cfg.conv):
                ks = di // 128
                src_dram = h_full[l]
                tblA = src_dram[0:cfg.split_cap, :]
                tblB = src_dram[cfg.baseB:cfg.N, :]
                if l == 2:
                    pool_ps = psP.tile([128, G], F32, tag="pool")
                gpb = max(1, 512 // (ks * TS))  # groups per PSUM batch (1-bank scratch)
                for t in range(T):
                    cb = int(tile_base[t])
                    Ct = int(nA_t[t] + nB_t[t])
                    rows = int(rows_t[t])
                    rows_ns = [min(128, rows), max(0, rows - 128)]
                    # groups: (r, [list of tile-local col indices])
                    groups = []
                    for r in range(NG):
                        cols = []
                        for h in range(2):
                            c0 = int(colstart[t, h, r]) - cb
                            cols.extend(range(c0, c0 + int(ncols[t, h, r])))
                        if cols:
                            groups.append((r, cols))
                    g_sb = gp.tile([128, Ct * di], F16, tag="g")
                    CHUNK = 8  # columns per dma_gather call (ring-safe)
                    for half, tbl in ((0, tblA), (1, tblB)):
                        hc0 = 0 if half == 0 else int(nA_t[t])
                        hcn = int(nA_t[t]) if half == 0 else int(nB_t[t])
                        for q0 in range(0, hcn, CHUNK):
                            qn = min(CHUNK, hcn - q0)
                            c0 = hc0 + q0
                            n_idx = qn * 128
                            o0 = (cb + c0) * 8
                            nc.gpsimd.dma_gather(
                                out_ap=g_sb[:, c0 * di:(c0 + qn) * di].rearrange(
                                    "p (c j) -> p c j", j=di),
                                in_ap=tbl,
                                idxs_ap=idx_sb[:, o0:o0 + n_idx // 16],
                                num_idxs=n_idx,
                                num_idxs_reg=n_idx,
                                elem_size=di,
                            )
                    h_all = hp.tile([128, Ct * TS], F16, tag="h")
                    nc.vector.tensor_tensor(
                        out=h_all[:].rearrange("p (c j) -> p c j", j=TS),
                        in0=iota_sb[:, :Ct * TS].rearrange("p (c j) -> p c j", j=TS),
                        in1=dstl_sb[:, cb:cb + Ct, None].to_broadcast([128, Ct, TS]),
                        op=mybir.AluOpType.is_equal,
                    )
                    agg = psG.tile([128, 1024], F32, tag="agg")
                    for ns in range(2):
                        if rows_ns[ns] > 0:
                            nc.tensor.matmul(
                                out=agg[:, ns * 512:ns * 512 + do],
                                lhsT=ones_sb[:1, :], rhs=bsb[l][:1, :],
                                start=True, stop=False,
                            )
                    batches = [groups[i:i + gpb] for i in range(0, len(groups), gpb)]
                    for bi, batch in enumerate(batches):
                        pa = psA.tile([128, 512], F32, tag="pa")
                        boff = 0
                        for (r, cols) in batch:
                            for k in range(ks):
                                for ci, col in enumerate(cols):
                                    nc.tensor.matmul(
                                        out=pa[:, boff + k * TS:boff + (k + 1) * TS],
                                        lhsT=g_sb[:, col * di + k * 128:
                                                  col * di + (k + 1) * 128],
                                        rhs=h_all[:, col * TS:(col + 1) * TS],
                                        start=(ci == 0), stop=(ci == len(cols) - 1),
                                    )
                            boff += ks * TS
                        at = atp.tile([128, 512], F16, tag="at")
                        nc.vector.tensor_copy(out=at[:, :boff], in_=pa[:, :boff])
                        boff = 0
                        for gi, (r, cols) in enumerate(batch):
                            for ns in range(2):
                                if rows_ns[ns] == 0:
                                    continue
                                for k in range(ks):
                                    last = (
                                        bi == len(batches) - 1
                                        and gi == len(batch) - 1
                                        and k == ks - 1
                                    )
                                    nc.tensor.matmul(
                                        out=agg[:, ns * 512:ns * 512 + do],
                                        lhsT=at[:, boff + k * TS + ns * 128:
                                                boff + k * TS + ns * 128 + 128],
                                        rhs=Wsb[l][k][:, r * do:(r + 1) * do],
                                        start=False, stop=last,
                                    )
                            boff += ks * TS
                    for ns in range(2):
                        rns = rows_ns[ns]
                        if rns == 0:
                            continue
                        st = t * 2 + ns
                        hn = hnp.tile([128, do], F16, tag="hn")
                        nc.scalar.activation(
                            out=hn[:], in_=agg[:, ns * 512:ns * 512 + do],
                            func=mybir.ActivationFunctionType.Relu,
                        )
                        if l < 2:
                            nc.sync.dma_start(
                                out=ag_in[l][st * 128:st * 128 + rns, :],
                                in_=hn[:rns, :],
                            )
                        else:
                            hg = mp.tile([128, G], F16, tag="hg")
                            nc.vector.tensor_tensor(
                                out=hg[:],
                                in0=iota_sb[:, :G],
                                in1=gid_sb[:, st:st + 1].to_broadcast([128, G]),
                                op=mybir.AluOpType.is_equal,
                            )
                            nc.vector.tensor_scalar_mul(
                                out=hg[:], in0=hg[:], scalar1=wnd_sb[:, st:st + 1]
                            )
                            nc.tensor.matmul(
                                out=pool_ps[:], lhsT=hn[:], rhs=hg[:],
                                start=(st == 0), stop=(st == ST - 1),
                            )
                if l < 2:
                    if timing:
                        nc.sync.dma_start(
                            out=h_full[l + 1][0:cfg.NPC, :], in_=ag_in[l][:, :]
                        )
                    else:
                        nc.gpsimd.collective_compute(
                            "AllGather",
                            mybir.AluOpType.bypass,
                            replica_groups=rg,
                            ins=[ag_in[l].ap().opt()],
                            outs=[h_full[l + 1].ap().opt()],
                        )
                    if debug:
                        nc.sync.dma_start(
                            out=dbg[f"h{l + 1}"][:, :], in_=h_full[l + 1][:, :]
                        )

            # ---- pooled AllReduce + MLP (transposed, fp32) ----
            pl_sb = mp.tile([128, G], F32, tag="pl")
            nc.vector.tensor_copy(out=pl_sb[:], in_=pool_ps[:])
            nc.sync.dma_start(out=pool_in[:, :], in_=pl_sb[:])
            if timing:
                nc.sync.dma_start(out=pool_out[:, :], in_=pool_in[:, :])
            else:
                nc.gpsimd.collective_compute(
                    "AllReduce",
                    mybir.AluOpType.add,
                    replica_groups=rg,
                    ins=[pool_in.ap().opt()],
                    outs=[pool_out.ap().opt()],
                )
            hgT = mp.tile([128, G], F32, tag="hgt")
            nc.sync.dma_start(out=hgT[:], in_=pool_out[:, :])
            if debug:
                nc.sync.dma_start(out=dbg["pool"][:, :], in_=pool_out[:, :])

            cur = [hgT]
            for l, (di, do) in enumerate(cfg.mlp):
                ks = di // 128
                nmt = -(-do // 128)
                nxt = []
                for mi in range(nmt):
                    mw = min(128, do - mi * 128)
                    ps = psG.tile([128, G], F32, tag="agg")
                    for k in range(ks):
                        nc.tensor.matmul(
                            out=ps[:mw, :],
                            lhsT=Wm_sb[l][k][:, mi * 128:mi * 128 + mw],
                            rhs=cur[k][:],
                            start=(k == 0), stop=(k == ks - 1),
                        )
                    hx = mp.tile([128, G], F32, tag=f"mlph{l}_{mi}")
                    nc.scalar.activation(
                        out=hx[:mw, :], in_=ps[:mw, :],
                        func=mybir.ActivationFunctionType.Relu,
                        bias=bm_sb[l][mi][:mw, :1],
                    )
                    nxt.append(hx)
                cur = nxt

            ps_cls = psG.tile([NC, G], F32, tag="agg")
            nc.tensor.matmul(
                out=ps_cls[:], lhsT=Wcls_sb[:, :NC], rhs=cur[0][:],
                start=True, stop=True,
            )
            lgT = mp.tile([NC, G], F32, tag="lgT")
            nc.vector.tensor_scalar_add(
                out=lgT[:], in0=ps_cls[:], scalar1=bcls_sb[:, :1]
            )
            ps_tr = psG.tile([G, NC], F32, tag="agg")
            nc.tensor.transpose(out=ps_tr[:], in_=lgT[:], identity=idc_sb[:])
            lg = mp.tile([G, NC], F32, tag="lg")
            nc.vector.tensor_copy(out=lg[:], in_=ps_tr[:])
            if debug:
                nc.sync.dma_start(out=dbg["lg"][:, :], in_=lg[:])
            mx = mp.tile([G, 1], F32, tag="mx")
            nc.vector.tensor_reduce(
                out=mx[:], in_=lg[:], axis=mybir.AxisListType.X,
                op=mybir.AluOpType.max,
            )
            nc.vector.tensor_scalar_mul(out=mx[:], in0=mx[:], scalar1=-1.0)
            ex = mp.tile([G, NC], F32, tag="ex")
            nc.scalar.activation(
                out=ex[:], in_=lg[:], func=mybir.ActivationFunctionType.Exp,
                bias=mx[:, :1],
            )
            sm = mp.tile([G, 1], F32, tag="sm")
            nc.vector.tensor_reduce(
                out=sm[:], in_=ex[:], axis=mybir.AxisListType.X,
                op=mybir.AluOpType.add,
            )
            rs = mp.tile([G, 1], F32, tag="rs")
            nc.vector.reciprocal(out=rs[:], in_=sm[:])
            ot = mp.tile([G, NC], F32, tag="ot")
            nc.vector.tensor_scalar_mul(out=ot[:], in0=ex[:], scalar1=rs[:, :1])
            nc.sync.dma_start(out=outT[:, :], in_=ot[:])

    nc.compile()
    return nc


def make_in_maps(inputs, cfg, layout, idx_w, dstl_po):
    gid_po, wnd_po = _pool_arrays(
        np.asarray(inputs["graph_ids"]).astype(np.int64), cfg
    )
    packed = _pack_weights(inputs, cfg)
    Cmax = max(layout["Cmax"], 1)
    iota = np.tile(np.arange(TS, dtype=np.float16)[None, :], (128, Cmax))
    iota = iota.reshape(128, Cmax * TS)
    shared = {
        "h0": np.asarray(inputs["h"], np.float16),
        "iota": iota,
        "ones1": np.ones((1, 128), np.float16),
        "idc": np.eye(cfg.NC, dtype=np.float32),
    }
    shared.update(packed)
    in_maps = []
    for c in range(cfg.cores):
        m = dict(shared)
        m["idxw"] = idx_w[c]
        m["dstl"] = dstl_po[c]
        m["gid"] = gid_po[c]
        m["wnd"] = wnd_po[c]
        in_maps.append(m)
    return in_maps


_CACHE = {}
last_results = None


def _run(inputs, cfg, trace=False):
    global last_results
    src = np.asarray(inputs["src"]).astype(np.int64)
    dst = np.asarray(inputs["dst"]).astype(np.int64)
    rel = np.asarray(inputs["rel_types"]).astype(np.int64)
    layout, idx_w, dstl_po = _preprocess_edges(src, dst, rel, cfg)
    key = (cfg.N, layout["NCOL"], tuple(layout["ncols"].ravel().tolist()))
    if key not in _CACHE:
        _CACHE.clear()
        _CACHE[key] = build_program(cfg, layout)
    nc = _CACHE[key]
    in_maps = make_in_maps(inputs, cfg, layout, idx_w, dstl_po)
    res = bass_utils.run_bass_kernel_spmd(
        nc, in_maps, core_ids=list(range(cfg.cores)), trace=trace
    )
    last_results = res
    return res.results[0]["out"]


def kernel(**inputs):
    return _run(inputs, FULL_CFG, trace=False)


# revision 16
# speedup vs baseline: 1.2723x; 1.2723x over previous
"""RGCN (3 RelGraphConv layers + mean-pool + MLP + softmax) on 8 Trainium2 cores.

Strategy (dst-node sharding, data-parallel message passing):
  - Core c owns destination nodes [c*6250, (c+1)*6250), processed in 25 tiles
    of 256 nodes.
  - Host groups each core's incoming edges by (dst-tile, relation) plus a
    self-loop pseudo-relation (r=16, W_loop), and splits every group by source
    half (src < 32768 vs src >= 17232) into 128-slot columns so dma_gather's
    int16 indices can address the h table via two overlapping slices.
    The column layout (max over cores) is identical across cores -> one SPMD
    program serves all 8.
  - h is stored in fp16 (halves gather bandwidth); all PSUM accumulation is
    fp32.
  - Per dst tile: two dma_gather calls (A-table / B-table) fetch h[src] rows
    for all the tile's columns; a one-hot matrix H (DVE iota/is_equal, fp16)
    scatter-reduces via matmul A^T = G^T @ H per group into PSUM; after a
    PSUM->SBUF (cast to fp16) copy, agg += A @ W_r accumulates in PSUM
    (bias via rank-1 matmul); ReLU emits the core's h_next rows (fp16).
  - AllGather replicates h_next across cores between conv layers (2 total).
  - Layer-3 output feeds a weighted one-hot pooling matmul (weights 1/cnt
    host-computed) accumulated over all node subtiles -> [feat, graphs]
    partials; one AllReduce combines cores; the MLP runs transposed
    (h^T stays [feat, graphs]) in fp32; PE transpose + free-dim softmax
    produce the [128, 8] output (identical on every core).
"""

import sys

sys.path.insert(0, "/opt/trn_rl_repo")

import math
import numpy as np

from concourse import bass, bacc, mybir, tile
from concourse import bass_utils

F32 = mybir.dt.float32
F16 = mybir.dt.float16
I16 = mybir.dt.int16

TS = 256  # dst nodes per tile


class Cfg:
    def __init__(self, N, R, G, NC, cores, conv, mlp, split_cap=32768):
        self.N = N
        self.R = R
        self.G = G
        self.NC = NC
        self.cores = cores
        self.conv = conv
        self.mlp = mlp
        self.NPC = N // cores
        self.T = math.ceil(self.NPC / TS)
        self.split_cap = min(split_cap, N)  # table A = [0, split_cap)
        self.baseB = max(0, N - self.split_cap)  # table B = [baseB, N)
        # classification threshold: balance A/B group sizes while keeping
        # src < thr inside table A and src >= thr inside table B
        self.split_thr = min(max(N // 2, self.baseB), self.split_cap)


FULL_CFG = Cfg(
    N=50000, R=16, G=128, NC=8, cores=8,
    conv=[(128, 128), (128, 256), (256, 128)],
    mlp=[(128, 128), (128, 256), (256, 128)],
)


def _preprocess_edges(src, dst, rel, cfg):
    """Column layout (shared) + per-core slot arrays.

    Per tile column order: [A-cols of rel 0..R][B-cols of rel 0..R].
    Returns layout plus per-core idx (int16 table-local, 16-wrapped per call)
    and dstl (fp16 local dst id in tile, -1 pad).
    """
    C, NPC, T, R = cfg.cores, cfg.NPC, cfg.T, cfg.R
    NG = R + 1
    rows_t = np.minimum(NPC - np.arange(T) * TS, TS).astype(np.int64)

    owner = dst // NPC
    tloc = (dst % NPC) // TS
    isB = (src >= cfg.split_thr).astype(np.int64)
    # counts per (core, tile, half, rel)
    key_full = ((owner * T + tloc) * 2 + isB) * NG + rel
    cnt = np.bincount(key_full, minlength=C * T * 2 * NG).reshape(C, T, 2, NG)
    # self edges (r = R): node ids per tile are contiguous -> count per half
    for t in range(T):
        lo = t * TS
        for c in range(C):
            vs = c * NPC + lo + np.arange(int(rows_t[t]))
            nb = int((vs >= cfg.split_thr).sum())
            cnt[c, t, 0, R] = len(vs) - nb
            cnt[c, t, 1, R] = nb

    ncols = -(-cnt.max(axis=0) // 128)  # [T, 2, NG]
    # per-tile columns: A block then B block
    nA_t = ncols[:, 0, :].sum(axis=1)
    nB_t = ncols[:, 1, :].sum(axis=1)
    Ct_t = nA_t + nB_t
    tile_base = np.zeros(T + 1, np.int64)
    tile_base[1:] = np.cumsum(Ct_t)
    NCOL = int(tile_base[-1])

    # column start of each (t, half, rel) group-part
    colstart = np.zeros((T, 2, NG), np.int64)
    for t in range(T):
        cur = tile_base[t]
        for h in range(2):
            for r in range(NG):
                colstart[t, h, r] = cur
                cur += ncols[t, h, r]

    idx_po = np.zeros((C, 128, NCOL), np.int16)
    dstl_po = np.full((C, 128, NCOL), -1.0, np.float16)
    for c in range(C):
        m = owner == c
        es, ed, er = src[m], dst[m], rel[m]
        eb = (es >= cfg.split_thr).astype(np.int64)
        tl = (ed - c * NPC) // TS
        dl = (ed - c * NPC) % TS
        k = (tl * 2 + eb) * NG + er
        order = np.argsort(k, kind="stable")
        es, dl, k, eb = es[order], dl[order], k[order], eb[order]
        grp_start = np.searchsorted(k, np.arange(T * 2 * NG))
        j = np.arange(len(k)) - grp_start[k]
        kt, krem = k // (2 * NG), k % (2 * NG)
        kh, kr = krem // NG, krem % NG
        col = colstart[kt, kh, kr] + j // 128
        p = j % 128
        val = np.where(eb == 1, es - cfg.baseB, es).astype(np.int16)
        idx_po[c, p, col] = val
        dstl_po[c, p, col] = dl.astype(np.float16)
        # self edges
        for t in range(T):
            vl = np.arange(int(rows_t[t]))
            vg = c * NPC + t * TS + vl
            for h in range(2):
                sel = (vg >= cfg.split_thr) == (h == 1)
                if not sel.any():
                    continue
                vv, ll = vg[sel], vl[sel]
                cs = colstart[t, h, R]
                jj = np.arange(len(vv))
                vval = vv - cfg.baseB if h == 1 else vv
                idx_po[c, jj % 128, cs + jj // 128] = vval.astype(np.int16)
                dstl_po[c, jj % 128, cs + jj // 128] = ll.astype(np.float16)

    # 16-wrapped per-call index array: call (t, half) covers its column block
    TOT = NCOL * 128
    idx_w = np.zeros((C, 128, TOT // 16), np.int16)
    for c in range(C):
        for t in range(T):
            for h in range(2):
                if h == 0:
                    c0, ncol = tile_base[t], nA_t[t]
                else:
                    c0, ncol = tile_base[t] + nA_t[t], nB_t[t]
                if ncol == 0:
                    continue
                flat = idx_po[c, :, c0:c0 + ncol].T.ravel()  # j = col*128 + p
                blk = flat.reshape(-1, 16).T  # [16, n/16]
                o0 = int(c0) * 8  # c0*128/16
                idx_w[c, :, o0:o0 + len(flat) // 16] = np.tile(blk, (8, 1))

    layout = {
        "NCOL": NCOL,
        "ncols": ncols,            # [T, 2, NG]
        "colstart": colstart,      # [T, 2, NG]
        "tile_base": tile_base,
        "nA_t": nA_t, "nB_t": nB_t,
        "rows_t": rows_t,
        "Cmax": int(Ct_t.max()),
    }
    return layout, idx_w, dstl_po


def _pack_weights(inputs, cfg):
    R = cfg.R
    packed = {}
    for l, (di, do) in enumerate(cfg.conv):
        ks = di // 128
        Wp = np.zeros((ks, 128, (R + 1) * do), np.float16)
        Wr = np.asarray(inputs[f"W_rel{l}"], np.float32)
        Wl = np.asarray(inputs[f"W_loop{l}"], np.float32)
        for k in range(ks):
            for r in range(R):
                Wp[k, :, r * do:(r + 1) * do] = Wr[r, k * 128:(k + 1) * 128, :]
            Wp[k, :, R * do:(R + 1) * do] = Wl[k * 128:(k + 1) * 128, :]
        packed[f"Wcv{l}"] = Wp
        packed[f"bcv{l}"] = np.asarray(inputs[f"b{l}"], np.float16).reshape(1, do)
    for l, (di, do) in enumerate(cfg.mlp):
        ks = di // 128
        W = np.asarray(inputs[f"Wh{l}"], np.float32)
        packed[f"Wm{l}"] = W.reshape(ks, 128, do)
        nmt = -(-do // 128)
        bp = np.zeros((nmt, 128, 1), np.float32)
        b = np.asarray(inputs[f"bh{l}"], np.float32)
        for mi in range(nmt):
            seg = b[mi * 128:(mi + 1) * 128]
            bp[mi, :len(seg), 0] = seg
        packed[f"bm{l}"] = bp
    packed["Wcls"] = np.asarray(inputs["Wc"], np.float32).reshape(1, 128, cfg.NC)
    packed["bcls"] = np.asarray(inputs["bc"], np.float32).reshape(cfg.NC, 1)
    return packed


def _pool_arrays(graph_ids, cfg):
    """Per 128-node subtile: graph id and 1/cnt weight per node (fp16)."""
    C, NPC = cfg.cores, cfg.NPC
    ST = math.ceil(NPC / 128)
    cnts = np.bincount(graph_ids, minlength=cfg.G).astype(np.float64)
    wg = (1.0 / np.maximum(cnts, 1.0)).astype(np.float32)
    gid_po = np.full((C, 128, ST), -1.0, np.float16)
    wnd_po = np.zeros((C, 128, ST), np.float32)
    for c in range(C):
        for st in range(ST):
            nt = min(128, NPC - st * 128)
            if nt <= 0:
                continue
            v = c * NPC + st * 128 + np.arange(nt)
            gid_po[c, :nt, st] = graph_ids[v].astype(np.float16)
            wnd_po[c, :nt, st] = wg[graph_ids[v]]
    return gid_po, wnd_po


def build_program(cfg, layout, debug=False, timing=False):
    nc = bacc.Bacc(
        "TRN2", target_bir_lowering=False, debug=False,
        enable_asserts=False, num_devices=cfg.cores,
    )
    R, T, G, NC = cfg.R, cfg.T, cfg.G, cfg.NC
    NG = R + 1
    NCOL, Cmax = layout["NCOL"], layout["Cmax"]
    ncols, colstart = layout["ncols"], layout["colstart"]
    tile_base, nA_t, nB_t = layout["tile_base"], layout["nA_t"], layout["nB_t"]
    rows_t = layout["rows_t"]
    ST = math.ceil(cfg.NPC / 128)
    D0 = cfg.conv[0][0]

    h0 = nc.dram_tensor("h0", [cfg.N, D0], F16, kind="ExternalInput")
    idxT = nc.dram_tensor("idxw", [128, NCOL * 8], I16, kind="ExternalInput")
    dstlT = nc.dram_tensor("dstl", [128, NCOL], F16, kind="ExternalInput")
    gidT = nc.dram_tensor("gid", [128, ST], F16, kind="ExternalInput")
    wndT = nc.dram_tensor("wnd", [128, ST], F32, kind="ExternalInput")
    iotaT = nc.dram_tensor("iota", [128, max(Cmax, 1) * TS], F16, kind="ExternalInput")
    onesT = nc.dram_tensor("ones1", [1, 128], F16, kind="ExternalInput")
    idcT = nc.dram_tensor("idc", [NC, NC], F32, kind="ExternalInput")
    WcvT, bcvT = [], []
    for l, (di, do) in enumerate(cfg.conv):
        WcvT.append(nc.dram_tensor(f"Wcv{l}", [di // 128, 128, NG * do], F16,
                                   kind="ExternalInput"))
        bcvT.append(nc.dram_tensor(f"bcv{l}", [1, do], F16, kind="ExternalInput"))
    WmT, bmT = [], []
    for l, (di, do) in enumerate(cfg.mlp):
        WmT.append(nc.dram_tensor(f"Wm{l}", [di // 128, 128, do], F32,
                                  kind="ExternalInput"))
        bmT.append(nc.dram_tensor(f"bm{l}", [-(-do // 128), 128, 1], F32,
                                  kind="ExternalInput"))
    WclsT = nc.dram_tensor("Wcls", [1, 128, NC], F32, kind="ExternalInput")
    bclsT = nc.dram_tensor("bcls", [NC, 1], F32, kind="ExternalInput")
    outT = nc.dram_tensor("out", [G, NC], F32, kind="ExternalOutput")

    h_full = [h0]
    ag_in = []
    for l in range(2):
        do = cfg.conv[l][1]
        ag_in.append(nc.dram_tensor(f"agin{l}", [cfg.NPC, do], F16))
        h_full.append(nc.dram_tensor(f"hfull{l + 1}", [cfg.N, do], F16))
    pool_in = nc.dram_tensor("plin", [128, G], F32)
    pool_out = nc.dram_tensor("plout", [128, G], F32)
    dbg = {}
    if debug:
        dbg["h1"] = nc.dram_tensor("dbg_h1", [cfg.N, cfg.conv[0][1]], F16,
                                   kind="ExternalOutput")
        dbg["h2"] = nc.dram_tensor("dbg_h2", [cfg.N, cfg.conv[1][1]], F16,
                                   kind="ExternalOutput")
        dbg["pool"] = nc.dram_tensor("dbg_pool", [128, G], F32,
                                     kind="ExternalOutput")
        dbg["lg"] = nc.dram_tensor("dbg_lg", [G, NC], F32, kind="ExternalOutput")

    rg = [list(range(cfg.cores))]

    with tile.TileContext(nc) as tc:
        with (
            tc.tile_pool(name="const", bufs=1) as cp,
            tc.tile_pool(name="wp", bufs=1) as wp,
            tc.tile_pool(name="gp", bufs=2) as gp,
            tc.tile_pool(name="hp", bufs=2) as hp,
            tc.tile_pool(name="atp", bufs=2) as atp,
            tc.tile_pool(name="hnp", bufs=3) as hnp,
            tc.tile_pool(name="mp", bufs=2) as mp,
            tc.tile_pool(name="psA", bufs=2, space="PSUM") as psA,
            tc.tile_pool(name="psG", bufs=2, space="PSUM") as psG,
            tc.tile_pool(name="psP", bufs=1, space="PSUM") as psP,
        ):
            idx_sb = cp.tile([128, NCOL * 8], I16)
            nc.sync.dma_start(out=idx_sb[:], in_=idxT[:, :])
            dstl_sb = cp.tile([128, NCOL], F16)
            nc.sync.dma_start(out=dstl_sb[:], in_=dstlT[:, :])
            iota_sb = cp.tile([128, max(Cmax, 1) * TS], F16)
            nc.sync.dma_start(out=iota_sb[:], in_=iotaT[:, :])
            gid_sb = cp.tile([128, ST], F16)
            nc.sync.dma_start(out=gid_sb[:], in_=gidT[:, :])
            wnd_sb = cp.tile([128, ST], F32)
            nc.sync.dma_start(out=wnd_sb[:], in_=wndT[:, :])
            ones_sb = cp.tile([1, 128], F16)
            nc.sync.dma_start(out=ones_sb[:], in_=onesT[:, :])
            idc_sb = cp.tile([NC, NC], F32)
            nc.sync.dma_start(out=idc_sb[:], in_=idcT[:, :])

            Wsb, bsb = [], []
            for l, (di, do) in enumerate(cfg.conv):
                ks = di // 128
                Wk = []
                for k in range(ks):
                    w = wp.tile([128, NG * do], F16, tag=f"wcv{l}_{k}")
                    nc.sync.dma_start(out=w[:], in_=WcvT[l][k, :, :])
                    Wk.append(w)
                Wsb.append(Wk)
                b = wp.tile([1, do], F16, tag=f"bcv{l}")
                nc.sync.dma_start(out=b[:], in_=bcvT[l][:, :])
                bsb.append(b)
            Wm_sb, bm_sb = [], []
            for l, (di, do) in enumerate(cfg.mlp):
                ks = di // 128
                Wk = []
                for k in range(ks):
                    w = wp.tile([128, do], F32, tag=f"wm{l}_{k}")
                    nc.sync.dma_start(out=w[:], in_=WmT[l][k, :, :])
                    Wk.append(w)
                Wm_sb.append(Wk)
                nmt = -(-do // 128)
                bk = []
                for mi in range(nmt):
                    b = wp.tile([128, 1], F32, tag=f"bm{l}_{mi}")
                    nc.sync.dma_start(out=b[:], in_=bmT[l][mi, :, :])
                    bk.append(b)
                bm_sb.append(bk)
            Wcls_sb = wp.tile([128, NC], F32, tag="wcls")
            nc.sync.dma_start(out=Wcls_sb[:], in_=WclsT[0, :, :])
            bcls_sb = wp.tile([NC, 1], F32, tag="bcls")
            nc.sync.dma_start(out=bcls_sb[:], in_=bclsT[:, :])

            pool_ps = None

            for l, (di, do) in enumerate(cfg.conv):
                ks = di // 128
                src_dram = h_full[l]
                tblA = src_dram[0:cfg.split_cap, :]
                tblB = src_dram[cfg.baseB:cfg.N, :]
                if l == 2:
                    pool_ps = psP.tile([128, G], F32, tag="pool")
                gpb = max(1, 512 // (ks * TS))  # groups per PSUM batch (1-bank scratch)
                for t in range(T):
                    cb = int(tile_base[t])
                    Ct = int(nA_t[t] + nB_t[t])
                    rows = int(rows_t[t])
                    rows_ns = [min(128, rows), max(0, rows - 128)]
                    # groups: (r, [list of tile-local col indices])
                    groups = []
                    for r in range(NG):
                        cols = []
                        for h in range(2):
                            c0 = int(colstart[t, h, r]) - cb
                            cols.extend(range(c0, c0 + int(ncols[t, h, r])))
                        if cols:
                            groups.append((r, cols))
                    g_sb = gp.tile([128, Ct * di], F16, tag="g")
                    CHUNK = 8  # columns per dma_gather call (ring-safe)
                    for half, tbl in ((0, tblA), (1, tblB)):
                        hc0 = 0 if half == 0 else int(nA_t[t])
                        hcn = int(nA_t[t]) if half == 0 else int(nB_t[t])
                        for q0 in range(0, hcn, CHUNK):
                            qn = min(CHUNK, hcn - q0)
                            c0 = hc0 + q0
                            n_idx = qn * 128
                            o0 = (cb + c0) * 8
                            nc.gpsimd.dma_gather(
                                out_ap=g_sb[:, c0 * di:(c0 + qn) * di].rearrange(
                                    "p (c j) -> p c j", j=di),
                                in_ap=tbl,
                                idxs_ap=idx_sb[:, o0:o0 + n_idx // 16],
                                num_idxs=n_idx,
                                num_idxs_reg=n_idx,
                                elem_size=di,
                            )
                    h_all = hp.tile([128, Ct * TS], F16, tag="h")
                    nc.vector.tensor_tensor(
                        out=h_all[:].rearrange("p (c j) -> p c j", j=TS),
                        in0=iota_sb[:, :Ct * TS].rearrange("p (c j) -> p c j", j=TS),
                        in1=dstl_sb[:, cb:cb + Ct, None].to_broadcast([128, Ct, TS]),
                        op=mybir.AluOpType.is_equal,
                    )
                    agg = psG.tile([128, 1024], F32, tag="agg")
                    for ns in range(2):
                        if rows_ns[ns] > 0:
                            nc.tensor.matmul(
                                out=agg[:, ns * 512:ns * 512 + do],
                                lhsT=ones_sb[:1, :], rhs=bsb[l][:1, :],
                                start=True, stop=False,
                            )
                    batches = [groups[i:i + gpb] for i in range(0, len(groups), gpb)]
                    for bi, batch in enumerate(batches):
                        pa = psA.tile([128, 512], F32, tag="pa")
                        boff = 0
                        for (r, cols) in batch:
                            for k in range(ks):
                                for ci, col in enumerate(cols):
                                    nc.tensor.matmul(
                                        out=pa[:, boff + k * TS:boff + (k + 1) * TS],
                                        lhsT=g_sb[:, col * di + k * 128:
                                                  col * di + (k + 1) * 128],
                                        rhs=h_all[:, col * TS:(col + 1) * TS],
                                        start=(ci == 0), stop=(ci == len(cols) - 1),
                                    )
                            boff += ks * TS
                        at = atp.tile([128, 512], F16, tag="at")
                        nc.vector.tensor_copy(out=at[:, :boff], in_=pa[:, :boff])
                        boff = 0
                        for gi, (r, cols) in enumerate(batch):
                            for ns in range(2):
                                if rows_ns[ns] == 0:
                                    continue
                                for k in range(ks):
                                    last = (
                                        bi == len(batches) - 1
                                        and gi == len(batch) - 1
                                        and k == ks - 1
                                    )
                                    nc.tensor.matmul(
                                        out=agg[:, ns * 512:ns * 512 + do],
                                        lhsT=at[:, boff + k * TS + ns * 128:
                                                boff + k * TS + ns * 128 + 128],
                                        rhs=Wsb[l][k][:, r * do:(r + 1) * do],
                                        start=False, stop=last,
                                    )
                            boff += ks * TS
                    for ns in range(2):
                        rns = rows_ns[ns]
                        if rns == 0:
                            continue
                        st = t * 2 + ns
                        hn = hnp.tile([128, do], F16, tag="hn")
                        nc.scalar.activation(
                            out=hn[:], in_=agg[:, ns * 512:ns * 512 + do],
                            func=mybir.ActivationFunctionType.Relu,
                        )
                        if l < 2:
                            nc.sync.dma_start(
                                out=ag_in[l][st * 128:st * 128 + rns, :],
                                in_=hn[:rns, :],
                            )
                        else:
                            hg = mp.tile([128, G], F16, tag="hg")
                            nc.vector.tensor_tensor(
                                out=hg[:],
                                in0=iota_sb[:, :G],
                                in1=gid_sb[:, st:st + 1].to_broadcast([128, G]),
                                op=mybir.AluOpType.is_equal,
                            )
                            nc.vector.tensor_scalar_mul(
                                out=hg[:], in0=hg[:], scalar1=wnd_sb[:, st:st + 1]
                            )
                            nc.tensor.matmul(
                                out=pool_ps[:], lhsT=hn[:], rhs=hg[:],
                                start=(st == 0), stop=(st == ST - 1),
                            )
                if l < 2:
                    if timing:
                        nc.sync.dma_start(
                            out=h_full[l + 1][0:cfg.NPC, :], in_=ag_in[l][:, :]
                        )
                    else:
                        nc.gpsimd.collective_compute(
                            "AllGather",
                            mybir.AluOpType.bypass,
                            replica_groups=rg,
                            ins=[ag_in[l].ap().opt()],
                            outs=[h_full[l + 1].ap().opt()],
                        )
                    if debug:
                        nc.sync.dma_start(
                            out=dbg[f"h{l + 1}"][:, :], in_=h_full[l + 1][:, :]
                        )

            # ---- pooled AllReduce + MLP (transposed, fp32) ----
            pl_sb = mp.tile([128, G], F32, tag="pl")
            nc.vector.tensor_copy(out=pl_sb[:], in_=pool_ps[:])
            nc.sync.dma_start(out=pool_in[:, :], in_=pl_sb[:])
            if timing:
                nc.sync.dma_start(out=pool_out[:, :], in_=pool_in[:, :])
            else:
                nc.gpsimd.collective_compute(
                    "AllReduce",
                    mybir.AluOpType.add,
                    replica_groups=rg,
                    ins=[pool_in.ap().opt()],
                    outs=[pool_out.ap().opt()],
                )
            hgT = mp.tile([128, G], F32, tag="hgt")
            nc.sync.dma_start(out=hgT[:], in_=pool_out[:, :])
            if debug:
                nc.sync.dma_start(out=dbg["pool"][:, :], in_=pool_out[:, :])

            cur = [hgT]
            for l, (di, do) in enumerate(cfg.mlp):
                ks = di // 128
                nmt = -(-do // 128)
                nxt = []
                for mi in range(nmt):
                    mw = min(128, do - mi * 128)
                    ps = psG.tile([128, G], F32, tag="agg")
                    for k in range(ks):
                        nc.tensor.matmul(
                            out=ps[:mw, :],
                            lhsT=Wm_sb[l][k][:, mi * 128:mi * 128 + mw],
                            rhs=cur[k][:],
                            start=(k == 0), stop=(k == ks - 1),
                        )
                    hx = mp.tile([128, G], F32, tag=f"mlph{l}_{mi}")
                    nc.scalar.activation(
                        out=hx[:mw, :], in_=ps[:mw, :],
                        func=mybir.ActivationFunctionType.Relu,
                        bias=bm_sb[l][mi][:mw, :1],
                    )
                    nxt.append(hx)
                cur = nxt

            ps_cls = psG.tile([NC, G], F32, tag="agg")
            nc.tensor.matmul(
                out=ps_cls[:], lhsT=Wcls_sb[:, :NC], rhs=cur[0][:],
                start=True, stop=True,
            )
            lgT = mp.tile([NC, G], F32, tag="lgT")
            nc.vector.tensor_scalar_add(
                out=lgT[:], in0=ps_cls[:], scalar1=bcls_sb[:, :1]
            )
            ps_tr = psG.tile([G, NC], F32, tag="agg")
            nc.tensor.transpose(out=ps_tr[:], in_=lgT[:], identity=idc_sb[:])
            lg = mp.tile([G, NC], F32, tag="lg")
            nc.vector.tensor_copy(out=lg[:], in_=ps_tr[:])
            if debug:
                nc.sync.dma_start(out=dbg["lg"][:, :], in_=lg[:])
            mx = mp.tile([G, 1], F32, tag="mx")
            nc.vector.tensor_reduce(
                out=mx[:], in_=lg[:], axis=mybir.AxisListType.X,
                op=mybir.AluOpType.max,
            )
            nc.vector.tensor_scalar_mul(out=mx[:], in0=mx[:], scalar1=-1.0)
            ex = mp.tile([G, NC], F32, tag="ex")
            nc.scalar.activation(
                out=ex[:], in_=lg[:], func=mybir.ActivationFunctionType.Exp,
                bias=mx[:, :1],
            )
            sm = mp.tile([G, 1], F32, tag="sm")
            nc.vector.tensor_reduce(
                out=sm[:], in_=ex[:], axis=mybir.AxisListType.X,
                op=mybir.AluOpType.add,
            )
            rs = mp.tile([G, 1], F32, tag="rs")
            nc.vector.reciprocal(out=rs[:], in_=sm[:])
            ot = mp.tile([G, NC], F32, tag="ot")
            nc.vector.tensor_scalar_mul(out=ot[:], in0=ex[:], scalar1=rs[:, :1])
            nc.sync.dma_start(out=outT[:, :], in_=ot[:])

    nc.compile()
    return nc


def make_in_maps(inputs, cfg, layout, idx_w, dstl_po):
    gid_po, wnd_po = _pool_arrays(
        np.asarray(inputs["graph_ids"]).astype(np.int64), cfg
    )
    packed = _pack_weights(inputs, cfg)
    Cmax = max(layout["Cmax"], 1)
    iota = np.tile(np.arange(TS, dtype=np.float16)[None, :], (128, Cmax))
    iota = iota.reshape(128, Cmax * TS)
    shared = {
        "h0": np.asarray(inputs["h"], np.float16),
        "iota": iota,
        "ones1": np.ones((1, 128), np.float16),
        "idc": np.eye(cfg.NC, dtype=np.float32),
    }
    shared.update(packed)
    in_maps = []
    for c in range(cfg.cores):
        m = dict(shared)
        m["idxw"] = idx_w[c]
        m["dstl"] = dstl_po[c]
        m["gid"] = gid_po[c]
        m["wnd"] = wnd_po[c]
        in_maps.append(m)
    return in_maps


_CACHE = {}
last_results = None


def _run(inputs, cfg, trace=False):
    global last_results
    src = np.asarray(inputs["src"]).astype(np.int64)
    dst = np.asarray(inputs["dst"]).astype(np.int64)
    rel = np.asarray(inputs["rel_types"]).astype(np.int64)
    layout, idx_w, dstl_po = _preprocess_edges(src, dst, rel, cfg)
    key = (cfg.N, layout["NCOL"], tuple(layout["ncols"].ravel().tolist()))
    if key not in _CACHE:
        _CACHE.clear()
        _CACHE[key] = build_program(cfg, layout)
    nc = _CACHE[key]
    in_maps = make_in_maps(inputs, cfg, layout, idx_w, dstl_po)
    res = bass_utils.run_bass_kernel_spmd(
        nc, in_maps, core_ids=list(range(cfg.cores)), trace=trace
    )
    last_results = res
    return res.results[0]["out"]


def kernel(**inputs):
    return _run(inputs, FULL_CFG, trace=False)


# revision 17
# speedup vs baseline: 1.3619x; 1.0704x over previous
"""RGCN (3 RelGraphConv layers + mean-pool + MLP + softmax) on 8 Trainium2 cores.

Strategy (dst-node sharding, data-parallel message passing):
  - Core c owns destination nodes [c*6250, (c+1)*6250), processed in 25 tiles
    of 256 nodes.
  - Host groups each core's incoming edges by (dst-tile, relation) plus a
    self-loop pseudo-relation (r=16, W_loop), and splits every group by source
    half (src < 32768 vs src >= 17232) into 128-slot columns so dma_gather's
    int16 indices can address the h table via two overlapping slices.
    The column layout (max over cores) is identical across cores -> one SPMD
    program serves all 8.
  - h is stored in fp16 (halves gather bandwidth); all PSUM accumulation is
    fp32.
  - Per dst tile: two dma_gather calls (A-table / B-table) fetch h[src] rows
    for all the tile's columns; a one-hot matrix H (DVE iota/is_equal, fp16)
    scatter-reduces via matmul A^T = G^T @ H per group into PSUM; after a
    PSUM->SBUF (cast to fp16) copy, agg += A @ W_r accumulates in PSUM
    (bias via rank-1 matmul); ReLU emits the core's h_next rows (fp16).
  - AllGather replicates h_next across cores between conv layers (2 total).
  - Layer-3 output feeds a weighted one-hot pooling matmul (weights 1/cnt
    host-computed) accumulated over all node subtiles -> [feat, graphs]
    partials; one AllReduce combines cores; the MLP runs transposed
    (h^T stays [feat, graphs]) in fp32; PE transpose + free-dim softmax
    produce the [128, 8] output (identical on every core).
"""

import sys

sys.path.insert(0, "/opt/trn_rl_repo")

import math
import numpy as np

from concourse import bass, bacc, mybir, tile
from concourse import bass_utils

F32 = mybir.dt.float32
F16 = mybir.dt.float16
I16 = mybir.dt.int16

TS = 256  # dst nodes per tile


class Cfg:
    def __init__(self, N, R, G, NC, cores, conv, mlp, split_cap=32768):
        self.N = N
        self.R = R
        self.G = G
        self.NC = NC
        self.cores = cores
        self.conv = conv
        self.mlp = mlp
        self.NPC = N // cores
        self.T = math.ceil(self.NPC / TS)
        self.split_cap = min(split_cap, N)  # table A = [0, split_cap)
        self.baseB = max(0, N - self.split_cap)  # table B = [baseB, N)
        # classification threshold: balance A/B group sizes while keeping
        # src < thr inside table A and src >= thr inside table B
        self.split_thr = min(max(N // 2, self.baseB), self.split_cap)


FULL_CFG = Cfg(
    N=50000, R=16, G=128, NC=8, cores=8,
    conv=[(128, 128), (128, 256), (256, 128)],
    mlp=[(128, 128), (128, 256), (256, 128)],
)


def _preprocess_edges(src, dst, rel, cfg):
    """Column layout (shared) + per-core slot arrays.

    Per tile column order: [A-cols of rel 0..R][B-cols of rel 0..R].
    Returns layout plus per-core idx (int16 table-local, 16-wrapped per call)
    and dstl (fp16 local dst id in tile, -1 pad).
    """
    C, NPC, T, R = cfg.cores, cfg.NPC, cfg.T, cfg.R
    NG = R + 1
    rows_t = np.minimum(NPC - np.arange(T) * TS, TS).astype(np.int64)

    owner = dst // NPC
    tloc = (dst % NPC) // TS
    isB = (src >= cfg.split_thr).astype(np.int64)
    # counts per (core, tile, half, rel)
    key_full = ((owner * T + tloc) * 2 + isB) * NG + rel
    cnt = np.bincount(key_full, minlength=C * T * 2 * NG).reshape(C, T, 2, NG)
    # self edges (r = R): node ids per tile are contiguous -> count per half
    for t in range(T):
        lo = t * TS
        for c in range(C):
            vs = c * NPC + lo + np.arange(int(rows_t[t]))
            nb = int((vs >= cfg.split_thr).sum())
            cnt[c, t, 0, R] = len(vs) - nb
            cnt[c, t, 1, R] = nb

    ncols = -(-cnt.max(axis=0) // 128)  # [T, 2, NG]
    # per-tile columns: A block then B block
    nA_t = ncols[:, 0, :].sum(axis=1)
    nB_t = ncols[:, 1, :].sum(axis=1)
    Ct_t = nA_t + nB_t
    tile_base = np.zeros(T + 1, np.int64)
    tile_base[1:] = np.cumsum(Ct_t)
    NCOL = int(tile_base[-1])

    # column start of each (t, half, rel) group-part
    colstart = np.zeros((T, 2, NG), np.int64)
    for t in range(T):
        cur = tile_base[t]
        for h in range(2):
            for r in range(NG):
                colstart[t, h, r] = cur
                cur += ncols[t, h, r]

    idx_po = np.zeros((C, 128, NCOL), np.int16)
    dstl_po = np.full((C, 128, NCOL), -1.0, np.float16)
    for c in range(C):
        m = owner == c
        es, ed, er = src[m], dst[m], rel[m]
        eb = (es >= cfg.split_thr).astype(np.int64)
        tl = (ed - c * NPC) // TS
        dl = (ed - c * NPC) % TS
        k = (tl * 2 + eb) * NG + er
        order = np.argsort(k, kind="stable")
        es, dl, k, eb = es[order], dl[order], k[order], eb[order]
        grp_start = np.searchsorted(k, np.arange(T * 2 * NG))
        j = np.arange(len(k)) - grp_start[k]
        kt, krem = k // (2 * NG), k % (2 * NG)
        kh, kr = krem // NG, krem % NG
        col = colstart[kt, kh, kr] + j // 128
        p = j % 128
        val = np.where(eb == 1, es - cfg.baseB, es).astype(np.int16)
        idx_po[c, p, col] = val
        dstl_po[c, p, col] = dl.astype(np.float16)
        # self edges
        for t in range(T):
            vl = np.arange(int(rows_t[t]))
            vg = c * NPC + t * TS + vl
            for h in range(2):
                sel = (vg >= cfg.split_thr) == (h == 1)
                if not sel.any():
                    continue
                vv, ll = vg[sel], vl[sel]
                cs = colstart[t, h, R]
                jj = np.arange(len(vv))
                vval = vv - cfg.baseB if h == 1 else vv
                idx_po[c, jj % 128, cs + jj // 128] = vval.astype(np.int16)
                dstl_po[c, jj % 128, cs + jj // 128] = ll.astype(np.float16)

    # 16-wrapped per-call index array: call (t, half) covers its column block
    TOT = NCOL * 128
    idx_w = np.zeros((C, 128, TOT // 16), np.int16)
    for c in range(C):
        for t in range(T):
            for h in range(2):
                if h == 0:
                    c0, ncol = tile_base[t], nA_t[t]
                else:
                    c0, ncol = tile_base[t] + nA_t[t], nB_t[t]
                if ncol == 0:
                    continue
                flat = idx_po[c, :, c0:c0 + ncol].T.ravel()  # j = col*128 + p
                blk = flat.reshape(-1, 16).T  # [16, n/16]
                o0 = int(c0) * 8  # c0*128/16
                idx_w[c, :, o0:o0 + len(flat) // 16] = np.tile(blk, (8, 1))

    layout = {
        "NCOL": NCOL,
        "ncols": ncols,            # [T, 2, NG]
        "colstart": colstart,      # [T, 2, NG]
        "tile_base": tile_base,
        "nA_t": nA_t, "nB_t": nB_t,
        "rows_t": rows_t,
        "Cmax": int(Ct_t.max()),
    }
    return layout, idx_w, dstl_po


def _pack_weights(inputs, cfg):
    R = cfg.R
    packed = {}
    for l, (di, do) in enumerate(cfg.conv):
        ks = di // 128
        Wp = np.zeros((ks, 128, (R + 1) * do), np.float16)
        Wr = np.asarray(inputs[f"W_rel{l}"], np.float32)
        Wl = np.asarray(inputs[f"W_loop{l}"], np.float32)
        for k in range(ks):
            for r in range(R):
                Wp[k, :, r * do:(r + 1) * do] = Wr[r, k * 128:(k + 1) * 128, :]
            Wp[k, :, R * do:(R + 1) * do] = Wl[k * 128:(k + 1) * 128, :]
        packed[f"Wcv{l}"] = Wp
        packed[f"bcv{l}"] = np.asarray(inputs[f"b{l}"], np.float16).reshape(1, do)
    for l, (di, do) in enumerate(cfg.mlp):
        ks = di // 128
        W = np.asarray(inputs[f"Wh{l}"], np.float32)
        packed[f"Wm{l}"] = W.reshape(ks, 128, do)
        nmt = -(-do // 128)
        bp = np.zeros((nmt, 128, 1), np.float32)
        b = np.asarray(inputs[f"bh{l}"], np.float32)
        for mi in range(nmt):
            seg = b[mi * 128:(mi + 1) * 128]
            bp[mi, :len(seg), 0] = seg
        packed[f"bm{l}"] = bp
    packed["Wcls"] = np.asarray(inputs["Wc"], np.float32).reshape(1, 128, cfg.NC)
    packed["bcls"] = np.asarray(inputs["bc"], np.float32).reshape(cfg.NC, 1)
    return packed


def _pool_arrays(graph_ids, cfg):
    """Per 128-node subtile: graph id and 1/cnt weight per node (fp16)."""
    C, NPC = cfg.cores, cfg.NPC
    ST = math.ceil(NPC / 128)
    cnts = np.bincount(graph_ids, minlength=cfg.G).astype(np.float64)
    wg = (1.0 / np.maximum(cnts, 1.0)).astype(np.float32)
    gid_po = np.full((C, 128, ST), -1.0, np.float16)
    wnd_po = np.zeros((C, 128, ST), np.float32)
    for c in range(C):
        for st in range(ST):
            nt = min(128, NPC - st * 128)
            if nt <= 0:
                continue
            v = c * NPC + st * 128 + np.arange(nt)
            gid_po[c, :nt, st] = graph_ids[v].astype(np.float16)
            wnd_po[c, :nt, st] = wg[graph_ids[v]]
    return gid_po, wnd_po


def build_program(cfg, layout, debug=False, timing=False):
    nc = bacc.Bacc(
        "TRN2", target_bir_lowering=False, debug=False,
        enable_asserts=False, num_devices=cfg.cores,
    )
    R, T, G, NC = cfg.R, cfg.T, cfg.G, cfg.NC
    NG = R + 1
    NCOL, Cmax = layout["NCOL"], layout["Cmax"]
    ncols, colstart = layout["ncols"], layout["colstart"]
    tile_base, nA_t, nB_t = layout["tile_base"], layout["nA_t"], layout["nB_t"]
    rows_t = layout["rows_t"]
    ST = math.ceil(cfg.NPC / 128)
    D0 = cfg.conv[0][0]

    h0 = nc.dram_tensor("h0", [cfg.N, D0], F16, kind="ExternalInput")
    idxT = nc.dram_tensor("idxw", [128, NCOL * 8], I16, kind="ExternalInput")
    dstlT = nc.dram_tensor("dstl", [128, NCOL], F16, kind="ExternalInput")
    gidT = nc.dram_tensor("gid", [128, ST], F16, kind="ExternalInput")
    wndT = nc.dram_tensor("wnd", [128, ST], F32, kind="ExternalInput")
    iotaT = nc.dram_tensor("iota", [128, max(Cmax, 1) * TS], F16, kind="ExternalInput")
    onesT = nc.dram_tensor("ones1", [1, 128], F16, kind="ExternalInput")
    idcT = nc.dram_tensor("idc", [NC, NC], F32, kind="ExternalInput")
    WcvT, bcvT = [], []
    for l, (di, do) in enumerate(cfg.conv):
        WcvT.append(nc.dram_tensor(f"Wcv{l}", [di // 128, 128, NG * do], F16,
                                   kind="ExternalInput"))
        bcvT.append(nc.dram_tensor(f"bcv{l}", [1, do], F16, kind="ExternalInput"))
    WmT, bmT = [], []
    for l, (di, do) in enumerate(cfg.mlp):
        WmT.append(nc.dram_tensor(f"Wm{l}", [di // 128, 128, do], F32,
                                  kind="ExternalInput"))
        bmT.append(nc.dram_tensor(f"bm{l}", [-(-do // 128), 128, 1], F32,
                                  kind="ExternalInput"))
    WclsT = nc.dram_tensor("Wcls", [1, 128, NC], F32, kind="ExternalInput")
    bclsT = nc.dram_tensor("bcls", [NC, 1], F32, kind="ExternalInput")
    outT = nc.dram_tensor("out", [G, NC], F32, kind="ExternalOutput")

    h_full = [h0]
    ag_in = []
    for l in range(2):
        do = cfg.conv[l][1]
        ag_in.append(nc.dram_tensor(f"agin{l}", [cfg.NPC, do], F16))
        h_full.append(nc.dram_tensor(f"hfull{l + 1}", [cfg.N, do], F16))
    pool_in = nc.dram_tensor("plin", [128, G], F32)
    pool_out = nc.dram_tensor("plout", [128, G], F32)
    dbg = {}
    if debug:
        dbg["h1"] = nc.dram_tensor("dbg_h1", [cfg.N, cfg.conv[0][1]], F16,
                                   kind="ExternalOutput")
        dbg["h2"] = nc.dram_tensor("dbg_h2", [cfg.N, cfg.conv[1][1]], F16,
                                   kind="ExternalOutput")
        dbg["pool"] = nc.dram_tensor("dbg_pool", [128, G], F32,
                                     kind="ExternalOutput")
        dbg["lg"] = nc.dram_tensor("dbg_lg", [G, NC], F32, kind="ExternalOutput")

    rg = [list(range(cfg.cores))]

    with tile.TileContext(nc) as tc:
        with (
            tc.tile_pool(name="const", bufs=1) as cp,
            tc.tile_pool(name="wp", bufs=1) as wp,
            tc.tile_pool(name="gp", bufs=2) as gp,
            tc.tile_pool(name="hp", bufs=3) as hp,
            tc.tile_pool(name="atp", bufs=4) as atp,
            tc.tile_pool(name="hnp", bufs=4) as hnp,
            tc.tile_pool(name="mp", bufs=2) as mp,
            tc.tile_pool(name="psA", bufs=3, space="PSUM") as psA,
            tc.tile_pool(name="psG", bufs=2, space="PSUM") as psG,
            tc.tile_pool(name="psP", bufs=1, space="PSUM") as psP,
        ):
            idx_sb = cp.tile([128, NCOL * 8], I16)
            nc.sync.dma_start(out=idx_sb[:], in_=idxT[:, :])
            dstl_sb = cp.tile([128, NCOL], F16)
            nc.sync.dma_start(out=dstl_sb[:], in_=dstlT[:, :])
            iota_sb = cp.tile([128, max(Cmax, 1) * TS], F16)
            nc.sync.dma_start(out=iota_sb[:], in_=iotaT[:, :])
            gid_sb = cp.tile([128, ST], F16)
            nc.sync.dma_start(out=gid_sb[:], in_=gidT[:, :])
            wnd_sb = cp.tile([128, ST], F32)
            nc.sync.dma_start(out=wnd_sb[:], in_=wndT[:, :])
            ones_sb = cp.tile([1, 128], F16)
            nc.sync.dma_start(out=ones_sb[:], in_=onesT[:, :])
            idc_sb = cp.tile([NC, NC], F32)
            nc.sync.dma_start(out=idc_sb[:], in_=idcT[:, :])

            Wsb, bsb = [], []
            for l, (di, do) in enumerate(cfg.conv):
                ks = di // 128
                Wk = []
                for k in range(ks):
                    w = wp.tile([128, NG * do], F16, tag=f"wcv{l}_{k}")
                    nc.sync.dma_start(out=w[:], in_=WcvT[l][k, :, :])
                    Wk.append(w)
                Wsb.append(Wk)
                b = wp.tile([1, do], F16, tag=f"bcv{l}")
                nc.sync.dma_start(out=b[:], in_=bcvT[l][:, :])
                bsb.append(b)
            Wm_sb, bm_sb = [], []
            for l, (di, do) in enumerate(cfg.mlp):
                ks = di // 128
                Wk = []
                for k in range(ks):
                    w = wp.tile([128, do], F32, tag=f"wm{l}_{k}")
                    nc.sync.dma_start(out=w[:], in_=WmT[l][k, :, :])
                    Wk.append(w)
                Wm_sb.append(Wk)
                nmt = -(-do // 128)
                bk = []
                for mi in range(nmt):
                    b = wp.tile([128, 1], F32, tag=f"bm{l}_{mi}")
                    nc.sync.dma_start(out=b[:], in_=bmT[l][mi, :, :])
                    bk.append(b)
                bm_sb.append(bk)
            Wcls_sb = wp.tile([128, NC], F32, tag="wcls")
            nc.sync.dma_start(out=Wcls_sb[:], in_=WclsT[0, :, :])
            bcls_sb = wp.tile([NC, 1], F32, tag="bcls")
            nc.sync.dma_start(out=bcls_sb[:], in_=bclsT[:, :])

            pool_ps = None

            for l, (di, do) in enumerate(cfg.conv):
                ks = di // 128
                src_dram = h_full[l]
                tblA = src_dram[0:cfg.split_cap, :]
                tblB = src_dram[cfg.baseB:cfg.N, :]
                if l == 2:
                    pool_ps = psP.tile([128, G], F32, tag="pool")
                gpb = max(1, 512 // (ks * TS))  # groups per PSUM batch (1-bank scratch)
                for t in range(T):
                    cb = int(tile_base[t])
                    Ct = int(nA_t[t] + nB_t[t])
                    rows = int(rows_t[t])
                    rows_ns = [min(128, rows), max(0, rows - 128)]
                    # groups: (r, [list of tile-local col indices])
                    groups = []
                    for r in range(NG):
                        cols = []
                        for h in range(2):
                            c0 = int(colstart[t, h, r]) - cb
                            cols.extend(range(c0, c0 + int(ncols[t, h, r])))
                        if cols:
                            groups.append((r, cols))
                    g_sb = gp.tile([128, Ct * di], F16, tag="g")
                    CHUNK = 8  # columns per dma_gather call (ring-safe)
                    for half, tbl in ((0, tblA), (1, tblB)):
                        hc0 = 0 if half == 0 else int(nA_t[t])
                        hcn = int(nA_t[t]) if half == 0 else int(nB_t[t])
                        for q0 in range(0, hcn, CHUNK):
                            qn = min(CHUNK, hcn - q0)
                            c0 = hc0 + q0
                            n_idx = qn * 128
                            o0 = (cb + c0) * 8
                            nc.gpsimd.dma_gather(
                                out_ap=g_sb[:, c0 * di:(c0 + qn) * di].rearrange(
                                    "p (c j) -> p c j", j=di),
                                in_ap=tbl,
                                idxs_ap=idx_sb[:, o0:o0 + n_idx // 16],
                                num_idxs=n_idx,
                                num_idxs_reg=n_idx,
                                elem_size=di,
                            )
                    h_all = hp.tile([128, Ct * TS], F16, tag="h")
                    nc.vector.tensor_tensor(
                        out=h_all[:].rearrange("p (c j) -> p c j", j=TS),
                        in0=iota_sb[:, :Ct * TS].rearrange("p (c j) -> p c j", j=TS),
                        in1=dstl_sb[:, cb:cb + Ct, None].to_broadcast([128, Ct, TS]),
                        op=mybir.AluOpType.is_equal,
                    )
                    agg = psG.tile([128, 1024], F32, tag="agg")
                    for ns in range(2):
                        if rows_ns[ns] > 0:
                            nc.tensor.matmul(
                                out=agg[:, ns * 512:ns * 512 + do],
                                lhsT=ones_sb[:1, :], rhs=bsb[l][:1, :],
                                start=True, stop=False,
                            )
                    batches = [groups[i:i + gpb] for i in range(0, len(groups), gpb)]
                    for bi, batch in enumerate(batches):
                        pa = psA.tile([128, 512], F32, tag="pa")
                        boff = 0
                        for (r, cols) in batch:
                            for k in range(ks):
                                for ci, col in enumerate(cols):
                                    nc.tensor.matmul(
                                        out=pa[:, boff + k * TS:boff + (k + 1) * TS],
                                        lhsT=g_sb[:, col * di + k * 128:
                                                  col * di + (k + 1) * 128],
                                        rhs=h_all[:, col * TS:(col + 1) * TS],
                                        start=(ci == 0), stop=(ci == len(cols) - 1),
                                    )
                            boff += ks * TS
                        at = atp.tile([128, 512], F16, tag="at")
                        nc.vector.tensor_copy(out=at[:, :boff], in_=pa[:, :boff])
                        boff = 0
                        for gi, (r, cols) in enumerate(batch):
                            for ns in range(2):
                                if rows_ns[ns] == 0:
                                    continue
                                for k in range(ks):
                                    last = (
                                        bi == len(batches) - 1
                                        and gi == len(batch) - 1
                                        and k == ks - 1
                                    )
                                    nc.tensor.matmul(
                                        out=agg[:, ns * 512:ns * 512 + do],
                                        lhsT=at[:, boff + k * TS + ns * 128:
                                                boff + k * TS + ns * 128 + 128],
                                        rhs=Wsb[l][k][:, r * do:(r + 1) * do],
                                        start=False, stop=last,
                                    )
                            boff += ks * TS
                    for ns in range(2):
                        rns = rows_ns[ns]
                        if rns == 0:
                            continue
                        st = t * 2 + ns
                        hn = hnp.tile([128, do], F16, tag="hn")
                        nc.scalar.activation(
                            out=hn[:], in_=agg[:, ns * 512:ns * 512 + do],
                            func=mybir.ActivationFunctionType.Relu,
                        )
                        if l < 2:
                            nc.sync.dma_start(
                                out=ag_in[l][st * 128:st * 128 + rns, :],
                                in_=hn[:rns, :],
                            )
                        else:
                            hg = mp.tile([128, G], F16, tag="hg")
                            nc.vector.tensor_tensor(
                                out=hg[:],
                                in0=iota_sb[:, :G],
                                in1=gid_sb[:, st:st + 1].to_broadcast([128, G]),
                                op=mybir.AluOpType.is_equal,
                            )
                            nc.vector.tensor_scalar_mul(
                                out=hg[:], in0=hg[:], scalar1=wnd_sb[:, st:st + 1]
                            )
                            nc.tensor.matmul(
                                out=pool_ps[:], lhsT=hn[:], rhs=hg[:],
                                start=(st == 0), stop=(st == ST - 1),
                            )
                if l < 2:
                    if timing:
                        nc.sync.dma_start(
                            out=h_full[l + 1][0:cfg.NPC, :], in_=ag_in[l][:, :]
                        )
                    else:
                        nc.gpsimd.collective_compute(
                            "AllGather",
                            mybir.AluOpType.bypass,
                            replica_groups=rg,
                            ins=[ag_in[l].ap().opt()],
                            outs=[h_full[l + 1].ap().opt()],
                        )
                    if debug:
                        nc.sync.dma_start(
                            out=dbg[f"h{l + 1}"][:, :], in_=h_full[l + 1][:, :]
                        )

            # ---- pooled AllReduce + MLP (transposed, fp32) ----
            pl_sb = mp.tile([128, G], F32, tag="pl")
            nc.vector.tensor_copy(out=pl_sb[:], in_=pool_ps[:])
            nc.sync.dma_start(out=pool_in[:, :], in_=pl_sb[:])
            if timing:
                nc.sync.dma_start(out=pool_out[:, :], in_=pool_in[:, :])
            else:
                nc.gpsimd.collective_compute(
                    "AllReduce",
                    mybir.AluOpType.add,
                    replica_groups=rg,
                    ins=[pool_in.ap().opt()],
                    outs=[pool_out.ap().opt()],
                )
            hgT = mp.tile([128, G], F32, tag="hgt")
            nc.sync.dma_start(out=hgT[:], in_=pool_out[:, :])
            if debug:
                nc.sync.dma_start(out=dbg["pool"][:, :], in_=pool_out[:, :])

            cur = [hgT]
            for l, (di, do) in enumerate(cfg.mlp):
                ks = di // 128
                nmt = -(-do // 128)
                nxt = []
                for mi in range(nmt):
                    mw = min(128, do - mi * 128)
                    ps = psG.tile([128, G], F32, tag="agg")
                    for k in range(ks):
                        nc.tensor.matmul(
                            out=ps[:mw, :],
                            lhsT=Wm_sb[l][k][:, mi * 128:mi * 128 + mw],
                            rhs=cur[k][:],
                            start=(k == 0), stop=(k == ks - 1),
                        )
                    hx = mp.tile([128, G], F32, tag=f"mlph{l}_{mi}")
                    nc.scalar.activation(
                        out=hx[:mw, :], in_=ps[:mw, :],
                        func=mybir.ActivationFunctionType.Relu,
                        bias=bm_sb[l][mi][:mw, :1],
                    )
                    nxt.append(hx)
                cur = nxt

            ps_cls = psG.tile([NC, G], F32, tag="agg")
            nc.tensor.matmul(
                out=ps_cls[:], lhsT=Wcls_sb[:, :NC], rhs=cur[0][:],
                start=True, stop=True,
            )
            lgT = mp.tile([NC, G], F32, tag="lgT")
            nc.vector.tensor_scalar_add(
                out=lgT[:], in0=ps_cls[:], scalar1=bcls_sb[:, :1]
            )
            ps_tr = psG.tile([G, NC], F32, tag="agg")
            nc.tensor.transpose(out=ps_tr[:], in_=lgT[:], identity=idc_sb[:])
            lg = mp.tile([G, NC], F32, tag="lg")
            nc.vector.tensor_copy(out=lg[:], in_=ps_tr[:])
            if debug:
                nc.sync.dma_start(out=dbg["lg"][:, :], in_=lg[:])
            mx = mp.tile([G, 1], F32, tag="mx")
            nc.vector.tensor_reduce(
                out=mx[:], in_=lg[:], axis=mybir.AxisListType.X,
                op=mybir.AluOpType.max,
            )
            nc.vector.tensor_scalar_mul(out=mx[:], in0=mx[:], scalar1=-1.0)
            ex = mp.tile([G, NC], F32, tag="ex")
            nc.scalar.activation(
                out=ex[:], in_=lg[:], func=mybir.ActivationFunctionType.Exp,
                bias=mx[:, :1],
            )
            sm = mp.tile([G, 1], F32, tag="sm")
            nc.vector.tensor_reduce(
                out=sm[:], in_=ex[:], axis=mybir.AxisListType.X,
                op=mybir.AluOpType.add,
            )
            rs = mp.tile([G, 1], F32, tag="rs")
            nc.vector.reciprocal(out=rs[:], in_=sm[:])
            ot = mp.tile([G, NC], F32, tag="ot")
            nc.vector.tensor_scalar_mul(out=ot[:], in0=ex[:], scalar1=rs[:, :1])
            nc.sync.dma_start(out=outT[:, :], in_=ot[:])

    nc.compile()
    return nc


def make_in_maps(inputs, cfg, layout, idx_w, dstl_po):
    gid_po, wnd_po = _pool_arrays(
        np.asarray(inputs["graph_ids"]).astype(np.int64), cfg
    )
    packed = _pack_weights(inputs, cfg)
    Cmax = max(layout["Cmax"], 1)
    iota = np.tile(np.arange(TS, dtype=np.float16)[None, :], (128, Cmax))
    iota = iota.reshape(128, Cmax * TS)
    shared = {
        "h0": np.asarray(inputs["h"], np.float16),
        "iota": iota,
        "ones1": np.ones((1, 128), np.float16),
        "idc": np.eye(cfg.NC, dtype=np.float32),
    }
    shared.update(packed)
    in_maps = []
    for c in range(cfg.cores):
        m = dict(shared)
        m["idxw"] = idx_w[c]
        m["dstl"] = dstl_po[c]
        m["gid"] = gid_po[c]
        m["wnd"] = wnd_po[c]
        in_maps.append(m)
    return in_maps


_CACHE = {}
last_results = None


def _run(inputs, cfg, trace=False):
    global last_results
    src = np.asarray(inputs["src"]).astype(np.int64)
    dst = np.asarray(inputs["dst"]).astype(np.int64)
    rel = np.asarray(inputs["rel_types"]).astype(np.int64)
    layout, idx_w, dstl_po = _preprocess_edges(src, dst, rel, cfg)
    key = (cfg.N, layout["NCOL"], tuple(layout["ncols"].ravel().tolist()))
    if key not in _CACHE:
        _CACHE.clear()
        _CACHE[key] = build_program(cfg, layout)
    nc = _CACHE[key]
    in_maps = make_in_maps(inputs, cfg, layout, idx_w, dstl_po)
    res = bass_utils.run_bass_kernel_spmd(
        nc, in_maps, core_ids=list(range(cfg.cores)), trace=trace
    )
    last_results = res
    return res.results[0]["out"]


def kernel(**inputs):
    return _run(inputs, FULL_CFG, trace=False)
